# revision 1
# baseline (speedup 1.0000x reference)
"""Trainium2 Bass kernel for nn_DeepSelfAttention_86346022518823.

Strategy (8 NeuronCores):
  - Shard the 200 independent QKV attention blocks 25-per-core (expert
    parallelism).  x is replicated (as x^T, bf16).
  - Per block b on each core:
      QT = Wq[b] @ x^T + bq   (computed directly transposed, [256, N])
      KT = Wk[b] @ x^T + bk
      V  = x @ Wv[b]^T        (natural layout [N, 256]; bv is skipped -- softmax
                               rows sum to 1 so the bias passes through linearly
                               and is re-added at the end as mean(bv))
      ST = KT^T @ QT          (scores *transposed*: [j, i])
      E  = exp(0.5*tanh(ST/(2*sqrt(D))) + 0.5)     == exp(sigmoid(ST/sqrt(D)))
           (tanh & exp share one ACT table set -> no table thrash)
      A[i, 0:257] += E[:, i]^T @ [V | 1]           (ones column gives the
                                                    softmax denominator free)
      acc[i, :] += A[:, :256] / A[:, 256]
  - ReduceScatter(sum) of acc over the 8 cores; each core gets a 512-row slice,
    scales by 1/200, adds mean(bv), transposes via the PE, and runs the
    20-layer MLP + final sigmoid layer on its slice in fp32.
  - Host concatenates the 8 row-slices.

All heavy matmuls are bf16 (fp32 PSUM accumulation); the MLP tail is fp32.
"""

import numpy as np
import ml_dtypes

import concourse.bass as bass
import concourse.mybir as mybir
import concourse.tile as tile
from concourse import bacc
from concourse.bass import ds
from concourse.bass_utils import run_bass_kernel_spmd
from concourse.masks import make_identity

BF16 = mybir.dt.bfloat16
F32 = mybir.dt.float32
AF = mybir.ActivationFunctionType
ALU = mybir.AluOpType

# problem constants (hardcoded per harness contract)
N = 4096
D = 256
NQKV = 200
NLAYERS = 20
NCORES = 8
P = 128
IG = 512           # i-group width (query columns per EV accumulation pass)
SGJ = 4            # j-tiles stacked per ST psum group (4 * 512 = 2048 free = 4 banks)
SCALE = 1.0 / 16.0  # 1/sqrt(D)
_FAKE_EXP_WITH_TANH = False  # timing experiment: same-set second ACT pass


def build(tc, io, n=N, bpc=NQKV // NCORES, ncores=NCORES, nlayers=NLAYERS,
          tail=True, unroll=False, timing_mode=False):
    """Emit the SPMD per-core program.  io maps tensor-name -> DRAM AP."""
    nc = tc.nc
    nqkv = bpc * ncores
    n_ic = n // P            # i-chunks (128-wide)
    n_ig = n // IG           # i-groups
    n_jt = n // P            # j-tiles
    n_sg = n_jt // SGJ       # stacked ST groups per i-group
    rpc = n // ncores        # rows per core after reduce-scatter
    n_rt = rpc // P          # 128-row tiles of the local slice
    Dp1 = D + 1

    xt_d, wq_d, wk_d, wv_d = io["xt"], io["wq"], io["wk"], io["wv"]
    bqk_d, wl_d, bld_d = io["bqk"], io["wl"], io["bld"]
    fw_d, fbias_d, meanbv_d, y_d = io["fw"], io["fbias"], io["meanbv"], io["y"]

    with (
        tc.tile_pool(name="persist", bufs=1) as persist,
        tc.tile_pool(name="dram", bufs=1, space="DRAM") as dram,
    ):
        # ---- persistent SBUF state ----
        xt0 = persist.tile([P, n], BF16)
        xt1 = persist.tile([P, n], BF16)
        # ping-pong projection state so adjacent blocks overlap
        qt_ab = [persist.tile([P, 2 * n], BF16, name=f"qt{i}") for i in range(2)]
        kt_ab = [persist.tile([P, 2 * n], BF16, name=f"kt{i}") for i in range(2)]
        vhat_ab = [
            persist.tile([P, n_jt * Dp1], BF16, name=f"vhat{i}") for i in range(2)
        ]
        acc = persist.tile([P, n_ic * D], F32)      # (ic, d)
        id_sb = persist.tile([P, P], F32)
        meanbv_sb = persist.tile([P, 2], F32)
        fbias_sb = persist.tile([P, 2], F32)
        half_sb = persist.tile([P, 1], F32)
        nc.vector.memset(half_sb, 0.5)

        nc.sync.dma_start(xt0, xt_d[0:P, :])
        nc.sync.dma_start(xt1, xt_d[P : 2 * P, :])
        nc.sync.dma_start(meanbv_sb, meanbv_d[:, :])
        nc.sync.dma_start(fbias_sb, fbias_d[:, :])
        make_identity(nc, id_sb)
        nc.vector.memset(acc, 0.0)
        for vh in vhat_ab:
            ones_view = vh.rearrange("p (j c) -> p j c", c=Dp1)[:, :, D : D + 1]
            nc.vector.memset(ones_view, 1.0)

        with (
            tc.tile_pool(name="work", bufs=2) as work,
            tc.tile_pool(name="ps_ev", bufs=4, space="PSUM") as ps_ev,
            tc.tile_pool(name="ps_st", bufs=1, space="PSUM") as ps_st,
        ):

            def block_body(bi, parity=0):
                qt_sb = qt_ab[parity]
                kt_sb = kt_ab[parity]
                vhat = vhat_ab[parity]
                # ---- per-block weight / bias loads ----
                if timing_mode:
                    bi = 0  # static offsets; stream shape identical
                wq_sb = work.tile([P, 2 * D], BF16, tag="wq", name="wq_sb")
                wk_sb = work.tile([P, 2 * D], BF16, tag="wk", name="wk_sb")
                wv_sb = work.tile([P, 2 * D], BF16, tag="wv", name="wv_sb")
                bqk_sb = work.tile([P, 4], F32, tag="bqk", name="bqk_sb")
                for kc in range(2):
                    nc.sync.dma_start(
                        wq_sb[:, kc * D : (kc + 1) * D], wq_d[ds(bi * D + kc * P, P), :]
                    )
                    nc.sync.dma_start(
                        wk_sb[:, kc * D : (kc + 1) * D], wk_d[ds(bi * D + kc * P, P), :]
                    )
                    nc.sync.dma_start(
                        wv_sb[:, kc * D : (kc + 1) * D], wv_d[ds(bi * D + kc * P, P), :]
                    )
                nc.sync.dma_start(bqk_sb, bqk_d[ds(bi * P, P), :])

                xts = (xt0, xt1)

                # ---- projections ----
                # Packed into ps_st-pool tiles (fast-recycling) so they never
                # contend with the long-lived EV accumulators in ps_ev.
                SGW_ = SGJ * IG
                qslots = SGW_ // IG   # QT/KT slices per psum tile
                vslots = SGW_ // D    # V slices per psum tile
                # QT / KT: [dout-chunk mc, i] = sum_kc W^T[kc, mc].T @ xT[kc]
                for w_sb, dst, qk in ((wq_sb, qt_sb, 0), (wk_sb, kt_sb, 1)):
                    for mc in range(2):
                        for icg in range(0, n_ig, qslots):
                            ps = ps_st.tile([P, SGW_], F32, tag="st", name="ps_proj")
                            cnt = min(qslots, n_ig - icg)
                            for sub in range(cnt):
                                ic = icg + sub
                                for kc in range(2):
                                    nc.tensor.matmul(
                                        ps[:, sub * IG : (sub + 1) * IG],
                                        w_sb[:, kc * D + mc * P : kc * D + mc * P + P],
                                        xts[kc][:, ic * IG : (ic + 1) * IG],
                                        start=(kc == 0),
                                        stop=(kc == 1),
                                    )
                            for sub in range(cnt):
                                ic = icg + sub
                                nc.vector.tensor_scalar(
                                    dst[:, mc * n + ic * IG : mc * n + (ic + 1) * IG],
                                    ps[:, sub * IG : (sub + 1) * IG],
                                    bqk_sb[:, qk * 2 + mc : qk * 2 + mc + 1],
                                    None,
                                    ALU.add,
                                )
                # V: [j-chunk, dout] = sum_kc xT[kc][:, jc].T @ WvT[kc]
                for jcg in range(0, n_jt, vslots):
                    ps = ps_st.tile([P, SGW_], F32, tag="st", name="ps_projv")
                    cnt = min(vslots, n_jt - jcg)
                    for sub in range(cnt):
                        jc = jcg + sub
                        for kc in range(2):
                            nc.tensor.matmul(
                                ps[:, sub * D : sub * D + D],
                                xts[kc][:, jc * P : (jc + 1) * P],
                                wv_sb[:, kc * D : (kc + 1) * D],
                                start=(kc == 0),
                                stop=(kc == 1),
                            )
                    for sub in range(cnt):
                        jc = jcg + sub
                        nc.vector.tensor_copy(
                            vhat[:, jc * Dp1 : jc * Dp1 + D],
                            ps[:, sub * D : sub * D + D],
                        )

                # ---- attention ----
                SGW = SGJ * IG  # free width of one stacked ST group
                for g in range(n_ig):
                    evas = [
                        ps_ev.tile([P, IG], F32, tag="ev", name=f"eva{c}")
                        for c in range(4)
                    ]
                    for sgp in range(n_sg // 2):
                        # two stacked ST groups share one wide exp pass
                        t_sb = work.tile([P, 2 * SGW], BF16, tag="t", name="t_sb")
                        e_sb = work.tile([P, 2 * SGW], BF16, tag="e", name="e_sb")
                        for half in range(2):
                            sg = sgp * 2 + half
                            stp = ps_st.tile([P, SGW], F32, tag="st", name="stp")
                            for jl in range(SGJ):
                                jc = sg * SGJ + jl
                                for kc in range(2):
                                    nc.tensor.matmul(
                                        stp[:, jl * IG : (jl + 1) * IG],
                                        kt_sb[:, kc * n + jc * P : kc * n + jc * P + P],
                                        qt_sb[:, kc * n + g * IG : kc * n + (g + 1) * IG],
                                        start=(kc == 0),
                                        stop=(kc == 1),
                                    )
                            nc.scalar.activation(
                                t_sb[:, half * SGW : (half + 1) * SGW],
                                stp,
                                AF.Tanh,
                                scale=SCALE / 2,
                            )
                        nc.scalar.activation(e_sb, t_sb, AF.Exp, bias=half_sb, scale=0.5)
                        for half in range(2):
                            sg = sgp * 2 + half
                            for jl in range(SGJ):
                                jc = sg * SGJ + jl
                                for c in range(4):
                                    nc.tensor.matmul(
                                        evas[c][:, :Dp1],
                                        e_sb[
                                            :,
                                            half * SGW + jl * IG + c * P : half * SGW
                                            + jl * IG
                                            + c * P
                                            + P,
                                        ],
                                        vhat[:, jc * Dp1 : (jc + 1) * Dp1],
                                        start=(jc == 0),
                                        stop=(jc == n_jt - 1),
                                    )
                    for c in range(4):
                        gc = g * 4 + c
                        r_sb = work.tile([P, 1], F32, tag="r", name="r_sb")
                        nc.vector.reciprocal(r_sb, evas[c][:, D : D + 1])
                        tmp = work.tile([P, D], F32, tag="tmp", name="tmp")
                        nc.vector.tensor_scalar(
                            tmp, evas[c][:, :D], r_sb, None, ALU.mult
                        )
                        nc.vector.tensor_tensor(
                            acc[:, gc * D : (gc + 1) * D],
                            acc[:, gc * D : (gc + 1) * D],
                            tmp,
                            ALU.add,
                        )

            hints = (
                mybir.EngineType.PE,
                mybir.EngineType.Activation,
                mybir.EngineType.DVE,
            )
            if timing_mode:
                reps_sb = persist.tile([1, 1], mybir.dt.int32, name="reps_sb")
                nc.sync.dma_start(reps_sb, io["reps"][0:1, 0:1])
                rv = nc.values_load(
                    reps_sb[0:1, 0:1],
                    min_val=1,
                    max_val=100000,
                    skip_runtime_bounds_check=True,
                )
                with tc.For_i(0, rv, 1, hint_engines=hints) as bi:
                    block_body(bi, 0)
                    block_body(bi, 1)
            elif unroll:
                for b in range(bpc):
                    block_body(b, b % 2)
            elif bpc > 2:
                pairs = bpc // 2
                with tc.For_i(0, 2 * pairs, 2, hint_engines=hints) as bi:
                    block_body(bi, 0)
                    block_body(bi + 1, 1)
                for b in range(2 * pairs, bpc):
                    block_body(b, 0)
            else:
                for b in range(bpc):
                    block_body(b, b % 2)

        # ---- reduce-scatter over cores ----
        ar_in = dram.tile([n, D], F32, name="ar_in")
        rs_out = dram.tile([rpc, D], F32, name="rs_out")
        nc.sync.dma_start(
            ar_in[:, :].rearrange("(gc p) d -> p gc d", p=P),
            acc.rearrange("p (gc d) -> p gc d", d=D),
        )
        if not tail:
            # profiling variant: no collective / MLP; dump an acc slice as y
            nc.sync.dma_start(
                y_d[:, :].rearrange("(c p) r -> p c r", p=P),
                acc[:, : 2 * rpc].rearrange("p (c r) -> p c r", r=rpc),
            )
            return
        nc.gpsimd.collective_compute(
            "ReduceScatter",
            ALU.add,
            ins=[ar_in.opt()],
            outs=[rs_out.opt()],
            replica_groups=[list(range(ncores))],
        )

        # ---- tail: transpose slice, MLP, final layer ----
        with (
            tc.tile_pool(name="tail", bufs=2) as tail,
            tc.tile_pool(name="ps_tail", bufs=4, space="PSUM") as ps_tail,
        ):
            rs_sb = tail.tile([P, n_rt * D], F32, name="rs_sb", bufs=1)
            nc.sync.dma_start(
                rs_sb.rearrange("p (t d) -> p t d", d=D),
                rs_out[:, :].rearrange("(t p) d -> p t d", p=P),
            )
            ht_a = [
                tail.tile([P, rpc], F32, name=f"hta{dc}", bufs=1) for dc in range(2)
            ]
            ht_b = [
                tail.tile([P, rpc], F32, name=f"htb{dc}", bufs=1) for dc in range(2)
            ]
            for dc in range(2):
                for t in range(n_rt):
                    pst = ps_tail.tile([P, IG], F32, tag="pt", name="pst")
                    nc.tensor.transpose(
                        pst[:, :P], rs_sb[:, t * D + dc * P : t * D + dc * P + P], id_sb
                    )
                    nc.vector.tensor_scalar(
                        ht_a[dc][:, t * P : (t + 1) * P],
                        pst[:, :P],
                        1.0 / nqkv,
                        meanbv_sb[:, dc : dc + 1],
                        ALU.mult,
                        ALU.add,
                    )
            h_cur, h_nxt = ht_a, ht_b
            for l in range(nlayers):
                wl_sb = tail.tile([P, 2 * D], F32, tag="wl", name="wl_sb")
                bl_sb = tail.tile([P, 2], F32, tag="bl", name="bl_sb")
                for kc in range(2):
                    nc.sync.dma_start(
                        wl_sb[:, kc * D : (kc + 1) * D],
                        wl_d[l * D + kc * P : l * D + kc * P + P, :],
                    )
                nc.sync.dma_start(bl_sb, bld_d[l * P : (l + 1) * P, :])
                for dc in range(2):
                    ps = ps_tail.tile([P, IG], F32, tag="pt", name="ps_mlp")
                    for kc in range(2):
                        nc.tensor.matmul(
                            ps[:, :rpc],
                            wl_sb[:, kc * D + dc * P : kc * D + dc * P + P],
                            h_cur[kc],
                            start=(kc == 0),
                            stop=(kc == 1),
                        )
                    nc.vector.tensor_scalar(
                        h_nxt[dc],
                        ps[:, :rpc],
                        bl_sb[:, dc : dc + 1],
                        0.0,
                        ALU.add,
                        ALU.max,
                    )
                h_cur, h_nxt = h_nxt, h_cur
            # final: y = sigmoid(h^T @ fw)^T + fb
            fw_sb = tail.tile([P, 2 * D], F32, name="fw_sb", bufs=1)
            for kc in range(2):
                nc.sync.dma_start(
                    fw_sb[:, kc * D : (kc + 1) * D], fw_d[kc * P : (kc + 1) * P, :]
                )
            for dc in range(2):
                ps = ps_tail.tile([P, IG], F32, tag="pt", name="ps_fin")
                for kc in range(2):
                    nc.tensor.matmul(
                        ps[:, :rpc],
                        fw_sb[:, kc * D + dc * P : kc * D + dc * P + P],
                        h_cur[kc],
                        start=(kc == 0),
                        stop=(kc == 1),
                    )
                sig = tail.tile([P, rpc], F32, tag="sig", name="sig")
                nc.scalar.activation(sig, ps[:, :rpc], AF.Sigmoid)
                y_sb = tail.tile([P, rpc], F32, tag="ysb", name="y_sb")
                nc.vector.tensor_scalar(
                    y_sb, sig, fbias_sb[:, dc : dc + 1], None, ALU.add
                )
                nc.sync.dma_start(y_d[dc * P : (dc + 1) * P, :], y_sb)


def prep_inputs(x, Wq, bq, Wk, bk, Wv, bv, Wl, bl, final_weight, final_bias,
                ncores=NCORES):
    """Host-side sharding / layout prep.  Returns (in_maps, meta)."""
    bf16 = ml_dtypes.bfloat16
    n, d = x.shape
    nqkv = Wq.shape[0]
    bpc = nqkv // ncores
    nlayers = Wl.shape[0]

    xt = np.ascontiguousarray(x.T).astype(bf16)                    # [D, N]
    wl_t = np.ascontiguousarray(Wl.transpose(0, 2, 1)).reshape(nlayers * d, d).astype(np.float32)
    bld = np.ascontiguousarray(bl.reshape(nlayers, 2, P).transpose(0, 2, 1)).reshape(nlayers * P, 2).astype(np.float32)
    fw = np.ascontiguousarray(final_weight).astype(np.float32)     # [D, D]
    fbias = np.ascontiguousarray(final_bias.reshape(2, P).T).astype(np.float32)
    meanbv = np.ascontiguousarray(bv.mean(axis=0).reshape(2, P).T).astype(np.float32)

    in_maps = []
    for c in range(ncores):
        sl = slice(c * bpc, (c + 1) * bpc)
        wq_t = np.ascontiguousarray(Wq[sl].transpose(0, 2, 1)).reshape(bpc * d, d).astype(bf16)
        wk_t = np.ascontiguousarray(Wk[sl].transpose(0, 2, 1)).reshape(bpc * d, d).astype(bf16)
        wv_t = np.ascontiguousarray(Wv[sl].transpose(0, 2, 1)).reshape(bpc * d, d).astype(bf16)
        # bqk[b*P + p, qk*2 + mc] = bias[qk][b, mc*P + p]
        bqk = np.empty((bpc * P, 4), np.float32)
        for qk, arr in ((0, bq[sl]), (1, bk[sl])):
            r = arr.reshape(bpc, 2, P)                             # [b, mc, p]
            bqk[:, qk * 2 : qk * 2 + 2] = r.transpose(0, 2, 1).reshape(bpc * P, 2)
        in_maps.append({
            "xt": xt, "wq": wq_t, "wk": wk_t, "wv": wv_t, "bqk": bqk,
            "wl": wl_t, "bld": bld, "fw": fw, "fbias": fbias, "meanbv": meanbv,
        })
    return in_maps


def build_program(n=N, bpc=NQKV // NCORES, ncores=NCORES, nlayers=NLAYERS,
                  tail=True, unroll=False, timing_mode=False):
    nc = bacc.Bacc("TRN2", target_bir_lowering=False, debug=False,
                   num_devices=ncores)
    d = D
    rpc = n // ncores
    io = {
        "xt": nc.dram_tensor("xt", [d, n], BF16, kind="ExternalInput").ap(),
        "wq": nc.dram_tensor("wq", [bpc * d, d], BF16, kind="ExternalInput").ap(),
        "wk": nc.dram_tensor("wk", [bpc * d, d], BF16, kind="ExternalInput").ap(),
        "wv": nc.dram_tensor("wv", [bpc * d, d], BF16, kind="ExternalInput").ap(),
        "bqk": nc.dram_tensor("bqk", [bpc * P, 4], F32, kind="ExternalInput").ap(),
        "wl": nc.dram_tensor("wl", [nlayers * d, d], F32, kind="ExternalInput").ap(),
        "bld": nc.dram_tensor("bld", [nlayers * P, 2], F32, kind="ExternalInput").ap(),
        "fw": nc.dram_tensor("fw", [d, d], F32, kind="ExternalInput").ap(),
        "fbias": nc.dram_tensor("fbias", [P, 2], F32, kind="ExternalInput").ap(),
        "meanbv": nc.dram_tensor("meanbv", [P, 2], F32, kind="ExternalInput").ap(),
        "y": nc.dram_tensor("y", [d, rpc], F32, kind="ExternalOutput").ap(),
    }
    if timing_mode:
        io["reps"] = nc.dram_tensor(
            "reps", [1, 1], mybir.dt.int32, kind="ExternalInput"
        ).ap()
    with tile.TileContext(nc) as tc:
        build(tc, io, n=n, bpc=bpc, ncores=ncores, nlayers=nlayers, tail=tail,
              unroll=unroll, timing_mode=timing_mode)
    nc.compile()
    return nc


_CACHED = {}


def kernel(**inputs):
    inputs = {k: np.asarray(v) for k, v in inputs.items()}
    x = inputs["x"]
    n = x.shape[0]
    nqkv = inputs["Wq"].shape[0]
    ncores = NCORES
    bpc = nqkv // ncores
    nlayers = inputs["Wl"].shape[0]
    rpc = n // ncores

    key = (n, bpc, ncores, nlayers)
    if key not in _CACHED:
        _CACHED[key] = build_program(n=n, bpc=bpc, ncores=ncores, nlayers=nlayers)
    nc = _CACHED[key]

    in_maps = prep_inputs(**inputs, ncores=ncores)
    res = run_bass_kernel_spmd(nc, in_maps, core_ids=list(range(ncores)))
    out = np.empty((n, D), np.float32)
    for c in range(ncores):
        out[c * rpc : (c + 1) * rpc, :] = res.results[c]["y"].T
    return out


if __name__ == "__main__":
    import reference

    inputs = reference.setup_inputs()
    out = kernel(**{k: np.asarray(v) for k, v in inputs.items()})
    print("out", out.shape, out.dtype)



# revision 5
# speedup vs baseline: 3.0500x; 3.0500x over previous
"""Trainium2 Bass kernel for nn_DeepSelfAttention_86346022518823.

Strategy (8 NeuronCores):
  - Shard the 200 independent QKV attention blocks 25-per-core (expert
    parallelism).  x is replicated (as x^T, bf16).
  - Per block b on each core:
      QT = Wq[b] @ x^T + bq   (computed directly transposed, [256, N])
      KT = Wk[b] @ x^T + bk
      V  = x @ Wv[b]^T        (natural layout [N, 256]; bv is skipped -- softmax
                               rows sum to 1 so the bias passes through linearly
                               and is re-added at the end as mean(bv))
      ST = KT^T @ QT          (scores *transposed*: [j, i])
      E  = exp(0.5*tanh(ST/(2*sqrt(D))) + 0.5)     == exp(sigmoid(ST/sqrt(D)))
           (tanh & exp share one ACT table set -> no table thrash)
      A[i, 0:257] += E[:, i]^T @ [V | 1]           (ones column gives the
                                                    softmax denominator free)
      acc[i, :] += A[:, :256] / A[:, 256]
  - ReduceScatter(sum) of acc over the 8 cores; each core gets a 512-row slice,
    scales by 1/200, adds mean(bv), transposes via the PE, and runs the
    20-layer MLP + final sigmoid layer on its slice in fp32.
  - Host concatenates the 8 row-slices.

All heavy matmuls are bf16 (fp32 PSUM accumulation); the MLP tail is fp32.
"""

import numpy as np
import ml_dtypes

import concourse.bass as bass
import concourse.mybir as mybir
import concourse.tile as tile
from concourse import bacc
from concourse.bass import ds
from concourse.bass_utils import run_bass_kernel_spmd
from concourse.masks import make_identity

BF16 = mybir.dt.bfloat16
F32 = mybir.dt.float32
AF = mybir.ActivationFunctionType
ALU = mybir.AluOpType

# problem constants (hardcoded per harness contract)
N = 4096
D = 256
NQKV = 200
NLAYERS = 20
NCORES = 8
P = 128
IG = 512           # i-group width (query columns per EV accumulation pass)
SGJ = 2            # j-tiles stacked per ST psum group (2 * 512 = 1024 free = 2 banks)
SCALE = 1.0 / 16.0  # 1/sqrt(D)
# exp(sigmoid(z)) ~= TA*tanh(TB*z + TC) + TD  (max rel err 4.1e-4 on [-12,12]).
# TA cancels in the softmax ratio; the TD term folds into the EV matmul as a
# rank-1 update with the column-sum of V-hat scaled by TD/TA.
TA = 0.85838554
TB = 0.50908561
TC = -0.24979685
TD = 1.85879184


def build(tc, io, n=N, bpc=NQKV // NCORES, ncores=NCORES, nlayers=NLAYERS,
          tail=True, unroll=False, timing_mode=False):
    """Emit the SPMD per-core program.  io maps tensor-name -> DRAM AP."""
    nc = tc.nc
    nqkv = bpc * ncores
    n_ic = n // P            # i-chunks (128-wide)
    n_ig = n // IG           # i-groups
    n_jt = n // P            # j-tiles
    n_sg = n_jt // SGJ       # stacked ST groups per i-group
    rpc = n // ncores        # rows per core after reduce-scatter
    n_rt = rpc // P          # 128-row tiles of the local slice
    Dp1 = D + 1

    xt_d, wq_d, wk_d, wv_d = io["xt"], io["wq"], io["wk"], io["wv"]
    bqk_d, wl_d, bld_d = io["bqk"], io["wl"], io["bld"]
    fw_d, fbias_d, meanbv_d, y_d = io["fw"], io["fbias"], io["meanbv"], io["y"]

    with (
        tc.tile_pool(name="persist", bufs=1) as persist,
        tc.tile_pool(name="dram", bufs=1, space="DRAM") as dram,
    ):
        # ---- persistent SBUF state ----
        xt0 = persist.tile([P, n], BF16)
        xt1 = persist.tile([P, n], BF16)
        # ping-pong projection state so adjacent blocks overlap
        qt_ab = [persist.tile([P, 2 * n], BF16, name=f"qt{i}") for i in range(2)]
        kt_ab = [persist.tile([P, 2 * n], BF16, name=f"kt{i}") for i in range(2)]
        vhat_ab = [
            persist.tile([P, n_jt * Dp1], BF16, name=f"vhat{i}") for i in range(2)
        ]
        acc = persist.tile([P, n_ic * D], F32)      # (ic, d)
        id_sb = persist.tile([P, P], F32)
        meanbv_sb = persist.tile([P, 2], F32)
        fbias_sb = persist.tile([P, 2], F32)
        tc_sb = persist.tile([P, 1], F32)
        ones_col = persist.tile([P, 1], BF16)
        ones_row = persist.tile([1, P], BF16)
        nc.vector.memset(tc_sb, TC)
        nc.vector.memset(ones_col, 1.0)
        nc.vector.memset(ones_row, 1.0)

        nc.sync.dma_start(xt0, xt_d[0:P, :])
        nc.sync.dma_start(xt1, xt_d[P : 2 * P, :])
        nc.sync.dma_start(meanbv_sb, meanbv_d[:, :])
        nc.sync.dma_start(fbias_sb, fbias_d[:, :])
        make_identity(nc, id_sb)
        nc.vector.memset(acc, 0.0)
        for vh in vhat_ab:
            ones_view = vh.rearrange("p (j c) -> p j c", c=Dp1)[:, :, D : D + 1]
            nc.vector.memset(ones_view, 1.0)

        with (
            tc.tile_pool(name="work", bufs=2) as work,
            tc.tile_pool(name="ps_ev", bufs=4, space="PSUM") as ps_ev,
            tc.tile_pool(name="ps_st", bufs=2, space="PSUM") as ps_st,
        ):

            def block_body(bi, parity=0):
                qt_sb = qt_ab[parity]
                kt_sb = kt_ab[parity]
                vhat = vhat_ab[parity]
                # ---- per-block weight / bias loads ----
                if timing_mode:
                    bi = 0  # static offsets; stream shape identical
                wq_sb = work.tile([P, 2 * D], BF16, tag="wq", name="wq_sb")
                wk_sb = work.tile([P, 2 * D], BF16, tag="wk", name="wk_sb")
                wv_sb = work.tile([P, 2 * D], BF16, tag="wv", name="wv_sb")
                bqk_sb = work.tile([P, 4], F32, tag="bqk", name="bqk_sb")
                for kc in range(2):
                    nc.sync.dma_start(
                        wq_sb[:, kc * D : (kc + 1) * D], wq_d[ds(bi * D + kc * P, P), :]
                    )
                    nc.sync.dma_start(
                        wk_sb[:, kc * D : (kc + 1) * D], wk_d[ds(bi * D + kc * P, P), :]
                    )
                    nc.sync.dma_start(
                        wv_sb[:, kc * D : (kc + 1) * D], wv_d[ds(bi * D + kc * P, P), :]
                    )
                nc.sync.dma_start(bqk_sb, bqk_d[ds(bi * P, P), :])

                xts = (xt0, xt1)

                # ---- projections ----
                # Packed into ps_st-pool tiles (fast-recycling) so they never
                # contend with the long-lived EV accumulators in ps_ev.
                SGW_ = SGJ * IG
                qslots = SGW_ // IG   # QT/KT slices per psum tile
                vslots = SGW_ // D    # V slices per psum tile
                # QT / KT: [dout-chunk mc, i] = sum_kc W^T[kc, mc].T @ xT[kc]
                for w_sb, dst, qk in ((wq_sb, qt_sb, 0), (wk_sb, kt_sb, 1)):
                    for mc in range(2):
                        for icg in range(0, n_ig, qslots):
                            ps = ps_st.tile([P, SGW_], F32, tag="st", name="ps_proj")
                            cnt = min(qslots, n_ig - icg)
                            for sub in range(cnt):
                                ic = icg + sub
                                for kc in range(2):
                                    nc.tensor.matmul(
                                        ps[:, sub * IG : (sub + 1) * IG],
                                        w_sb[:, kc * D + mc * P : kc * D + mc * P + P],
                                        xts[kc][:, ic * IG : (ic + 1) * IG],
                                        start=(kc == 0),
                                        stop=(kc == 1),
                                    )
                            for sub in range(cnt):
                                ic = icg + sub
                                nc.vector.tensor_scalar(
                                    dst[:, mc * n + ic * IG : mc * n + (ic + 1) * IG],
                                    ps[:, sub * IG : (sub + 1) * IG],
                                    bqk_sb[:, qk * 2 + mc : qk * 2 + mc + 1],
                                    None,
                                    ALU.add,
                                )
                # V: [j-chunk, dout] = sum_kc xT[kc][:, jc].T @ WvT[kc]
                for jcg in range(0, n_jt, vslots):
                    ps = ps_st.tile([P, SGW_], F32, tag="st", name="ps_projv")
                    cnt = min(vslots, n_jt - jcg)
                    for sub in range(cnt):
                        jc = jcg + sub
                        for kc in range(2):
                            nc.tensor.matmul(
                                ps[:, sub * D : sub * D + D],
                                xts[kc][:, jc * P : (jc + 1) * P],
                                wv_sb[:, kc * D : (kc + 1) * D],
                                start=(kc == 0),
                                stop=(kc == 1),
                            )
                    for sub in range(cnt):
                        jc = jcg + sub
                        nc.vector.tensor_copy(
                            vhat[:, jc * Dp1 : jc * Dp1 + D],
                            ps[:, sub * D : sub * D + D],
                        )

                # svrow = (TD/TA) * sum_j vhat_j  (rank-1 EV closer term)
                sv_ps = ps_ev.tile([P, IG], F32, tag="ev", name="sv_ps")
                for jc in range(n_jt):
                    nc.tensor.matmul(
                        sv_ps[0:1, :Dp1],
                        ones_col,
                        vhat[:, jc * Dp1 : (jc + 1) * Dp1],
                        start=(jc == 0),
                        stop=(jc == n_jt - 1),
                    )
                svrow = work.tile([1, Dp1], BF16, tag="sv", name="svrow")
                nc.vector.tensor_scalar(
                    svrow, sv_ps[0:1, :Dp1], TD / TA, None, ALU.mult
                )

                # ---- attention ----
                SGW = SGJ * IG  # free width of one stacked ST group
                for g in range(n_ig):
                    evas = [
                        ps_ev.tile([P, IG], F32, tag="ev", name=f"eva{c}")
                        for c in range(4)
                    ]
                    for sg in range(n_sg):
                        stp = ps_st.tile([P, SGW], F32, tag="st", name="stp")
                        for jl in range(SGJ):
                            jc = sg * SGJ + jl
                            for kc in range(2):
                                nc.tensor.matmul(
                                    stp[:, jl * IG : (jl + 1) * IG],
                                    kt_sb[:, kc * n + jc * P : kc * n + jc * P + P],
                                    qt_sb[:, kc * n + g * IG : kc * n + (g + 1) * IG],
                                    start=(kc == 0),
                                    stop=(kc == 1),
                                )
                        t_sb = work.tile([P, SGW], BF16, tag="t", name="t_sb")
                        nc.scalar.activation(
                            t_sb, stp, AF.Tanh, bias=tc_sb, scale=TB * SCALE
                        )
                        for jl in range(SGJ):
                            jc = sg * SGJ + jl
                            for c in range(4):
                                nc.tensor.matmul(
                                    evas[c][:, :Dp1],
                                    t_sb[:, jl * IG + c * P : jl * IG + c * P + P],
                                    vhat[:, jc * Dp1 : (jc + 1) * Dp1],
                                    start=(jc == 0),
                                    stop=False,
                                )
                    for c in range(4):
                        nc.tensor.matmul(
                            evas[c][:, :Dp1],
                            ones_row,
                            svrow,
                            start=False,
                            stop=True,
                        )
                    for c in range(4):
                        gc = g * 4 + c
                        r_sb = work.tile([P, 1], F32, tag="r", name="r_sb")
                        nc.vector.reciprocal(r_sb, evas[c][:, D : D + 1])
                        tmp = work.tile([P, D], F32, tag="tmp", name="tmp")
                        nc.vector.tensor_scalar(
                            tmp, evas[c][:, :D], r_sb, None, ALU.mult
                        )
                        nc.vector.tensor_tensor(
                            acc[:, gc * D : (gc + 1) * D],
                            acc[:, gc * D : (gc + 1) * D],
                            tmp,
                            ALU.add,
                        )

            hints = (
                mybir.EngineType.PE,
                mybir.EngineType.Activation,
                mybir.EngineType.DVE,
            )
            if timing_mode:
                reps_sb = persist.tile([1, 1], mybir.dt.int32, name="reps_sb")
                nc.sync.dma_start(reps_sb, io["reps"][0:1, 0:1])
                rv = nc.values_load(
                    reps_sb[0:1, 0:1],
                    min_val=1,
                    max_val=100000,
                    skip_runtime_bounds_check=True,
                )
                with tc.For_i(0, rv, 1, hint_engines=hints) as bi:
                    block_body(bi, 0)
                    block_body(bi, 1)
            elif unroll:
                for b in range(bpc):
                    block_body(b, b % 2)
            elif bpc > 2:
                pairs = bpc // 2
                with tc.For_i(0, 2 * pairs, 2, hint_engines=hints) as bi:
                    block_body(bi, 0)
                    block_body(bi + 1, 1)
                for b in range(2 * pairs, bpc):
                    block_body(b, 0)
            else:
                for b in range(bpc):
                    block_body(b, b % 2)

        # ---- reduce-scatter over cores ----
        ar_in = dram.tile([n, D], F32, name="ar_in")
        rs_out = dram.tile([rpc, D], F32, name="rs_out")
        nc.sync.dma_start(
            ar_in[:, :].rearrange("(gc p) d -> p gc d", p=P),
            acc.rearrange("p (gc d) -> p gc d", d=D),
        )
        if not tail:
            # profiling variant: no collective / MLP; dump an acc slice as y
            nc.sync.dma_start(
                y_d[:, :].rearrange("(c p) r -> p c r", p=P),
                acc[:, : 2 * rpc].rearrange("p (c r) -> p c r", r=rpc),
            )
            return
        nc.gpsimd.collective_compute(
            "ReduceScatter",
            ALU.add,
            ins=[ar_in.opt()],
            outs=[rs_out.opt()],
            replica_groups=[list(range(ncores))],
        )

        # ---- tail: transpose slice, MLP, final layer ----
        with (
            tc.tile_pool(name="tail", bufs=2) as tail,
            tc.tile_pool(name="ps_tail", bufs=4, space="PSUM") as ps_tail,
        ):
            rs_sb = tail.tile([P, n_rt * D], F32, name="rs_sb", bufs=1)
            nc.sync.dma_start(
                rs_sb.rearrange("p (t d) -> p t d", d=D),
                rs_out[:, :].rearrange("(t p) d -> p t d", p=P),
            )
            ht_a = [
                tail.tile([P, rpc], F32, name=f"hta{dc}", bufs=1) for dc in range(2)
            ]
            ht_b = [
                tail.tile([P, rpc], F32, name=f"htb{dc}", bufs=1) for dc in range(2)
            ]
            for dc in range(2):
                for t in range(n_rt):
                    pst = ps_tail.tile([P, IG], F32, tag="pt", name="pst")
                    nc.tensor.transpose(
                        pst[:, :P], rs_sb[:, t * D + dc * P : t * D + dc * P + P], id_sb
                    )
                    nc.vector.tensor_scalar(
                        ht_a[dc][:, t * P : (t + 1) * P],
                        pst[:, :P],
                        1.0 / nqkv,
                        meanbv_sb[:, dc : dc + 1],
                        ALU.mult,
                        ALU.add,
                    )
            h_cur, h_nxt = ht_a, ht_b
            for l in range(nlayers):
                wl_sb = tail.tile([P, 2 * D], F32, tag="wl", name="wl_sb")
                bl_sb = tail.tile([P, 2], F32, tag="bl", name="bl_sb")
                for kc in range(2):
                    nc.sync.dma_start(
                        wl_sb[:, kc * D : (kc + 1) * D],
                        wl_d[l * D + kc * P : l * D + kc * P + P, :],
                    )
                nc.sync.dma_start(bl_sb, bld_d[l * P : (l + 1) * P, :])
                for dc in range(2):
                    ps = ps_tail.tile([P, IG], F32, tag="pt", name="ps_mlp")
                    for kc in range(2):
                        nc.tensor.matmul(
                            ps[:, :rpc],
                            wl_sb[:, kc * D + dc * P : kc * D + dc * P + P],
                            h_cur[kc],
                            start=(kc == 0),
                            stop=(kc == 1),
                        )
                    nc.vector.tensor_scalar(
                        h_nxt[dc],
                        ps[:, :rpc],
                        bl_sb[:, dc : dc + 1],
                        0.0,
                        ALU.add,
                        ALU.max,
                    )
                h_cur, h_nxt = h_nxt, h_cur
            # final: y = sigmoid(h^T @ fw)^T + fb
            fw_sb = tail.tile([P, 2 * D], F32, name="fw_sb", bufs=1)
            for kc in range(2):
                nc.sync.dma_start(
                    fw_sb[:, kc * D : (kc + 1) * D], fw_d[kc * P : (kc + 1) * P, :]
                )
            for dc in range(2):
                ps = ps_tail.tile([P, IG], F32, tag="pt", name="ps_fin")
                for kc in range(2):
                    nc.tensor.matmul(
                        ps[:, :rpc],
                        fw_sb[:, kc * D + dc * P : kc * D + dc * P + P],
                        h_cur[kc],
                        start=(kc == 0),
                        stop=(kc == 1),
                    )
                sig = tail.tile([P, rpc], F32, tag="sig", name="sig")
                nc.scalar.activation(sig, ps[:, :rpc], AF.Sigmoid)
                y_sb = tail.tile([P, rpc], F32, tag="ysb", name="y_sb")
                nc.vector.tensor_scalar(
                    y_sb, sig, fbias_sb[:, dc : dc + 1], None, ALU.add
                )
                nc.sync.dma_start(y_d[dc * P : (dc + 1) * P, :], y_sb)


def prep_inputs(x, Wq, bq, Wk, bk, Wv, bv, Wl, bl, final_weight, final_bias,
                ncores=NCORES):
    """Host-side sharding / layout prep.  Returns (in_maps, meta)."""
    bf16 = ml_dtypes.bfloat16
    n, d = x.shape
    nqkv = Wq.shape[0]
    bpc = nqkv // ncores
    nlayers = Wl.shape[0]

    xt = np.ascontiguousarray(x.T).astype(bf16)                    # [D, N]
    wl_t = np.ascontiguousarray(Wl.transpose(0, 2, 1)).reshape(nlayers * d, d).astype(np.float32)
    bld = np.ascontiguousarray(bl.reshape(nlayers, 2, P).transpose(0, 2, 1)).reshape(nlayers * P, 2).astype(np.float32)
    fw = np.ascontiguousarray(final_weight).astype(np.float32)     # [D, D]
    fbias = np.ascontiguousarray(final_bias.reshape(2, P).T).astype(np.float32)
    meanbv = np.ascontiguousarray(bv.mean(axis=0).reshape(2, P).T).astype(np.float32)

    in_maps = []
    for c in range(ncores):
        sl = slice(c * bpc, (c + 1) * bpc)
        wq_t = np.ascontiguousarray(Wq[sl].transpose(0, 2, 1)).reshape(bpc * d, d).astype(bf16)
        wk_t = np.ascontiguousarray(Wk[sl].transpose(0, 2, 1)).reshape(bpc * d, d).astype(bf16)
        wv_t = np.ascontiguousarray(Wv[sl].transpose(0, 2, 1)).reshape(bpc * d, d).astype(bf16)
        # bqk[b*P + p, qk*2 + mc] = bias[qk][b, mc*P + p]
        bqk = np.empty((bpc * P, 4), np.float32)
        for qk, arr in ((0, bq[sl]), (1, bk[sl])):
            r = arr.reshape(bpc, 2, P)                             # [b, mc, p]
            bqk[:, qk * 2 : qk * 2 + 2] = r.transpose(0, 2, 1).reshape(bpc * P, 2)
        in_maps.append({
            "xt": xt, "wq": wq_t, "wk": wk_t, "wv": wv_t, "bqk": bqk,
            "wl": wl_t, "bld": bld, "fw": fw, "fbias": fbias, "meanbv": meanbv,
        })
    return in_maps


def build_program(n=N, bpc=NQKV // NCORES, ncores=NCORES, nlayers=NLAYERS,
                  tail=True, unroll=False, timing_mode=False):
    nc = bacc.Bacc("TRN2", target_bir_lowering=False, debug=False,
                   num_devices=ncores)
    d = D
    rpc = n // ncores
    io = {
        "xt": nc.dram_tensor("xt", [d, n], BF16, kind="ExternalInput").ap(),
        "wq": nc.dram_tensor("wq", [bpc * d, d], BF16, kind="ExternalInput").ap(),
        "wk": nc.dram_tensor("wk", [bpc * d, d], BF16, kind="ExternalInput").ap(),
        "wv": nc.dram_tensor("wv", [bpc * d, d], BF16, kind="ExternalInput").ap(),
        "bqk": nc.dram_tensor("bqk", [bpc * P, 4], F32, kind="ExternalInput").ap(),
        "wl": nc.dram_tensor("wl", [nlayers * d, d], F32, kind="ExternalInput").ap(),
        "bld": nc.dram_tensor("bld", [nlayers * P, 2], F32, kind="ExternalInput").ap(),
        "fw": nc.dram_tensor("fw", [d, d], F32, kind="ExternalInput").ap(),
        "fbias": nc.dram_tensor("fbias", [P, 2], F32, kind="ExternalInput").ap(),
        "meanbv": nc.dram_tensor("meanbv", [P, 2], F32, kind="ExternalInput").ap(),
        "y": nc.dram_tensor("y", [d, rpc], F32, kind="ExternalOutput").ap(),
    }
    if timing_mode:
        io["reps"] = nc.dram_tensor(
            "reps", [1, 1], mybir.dt.int32, kind="ExternalInput"
        ).ap()
    with tile.TileContext(nc) as tc:
        build(tc, io, n=n, bpc=bpc, ncores=ncores, nlayers=nlayers, tail=tail,
              unroll=unroll, timing_mode=timing_mode)
    nc.compile()
    return nc


_CACHED = {}


def kernel(**inputs):
    inputs = {k: np.asarray(v) for k, v in inputs.items()}
    x = inputs["x"]
    n = x.shape[0]
    nqkv = inputs["Wq"].shape[0]
    ncores = NCORES
    bpc = nqkv // ncores
    nlayers = inputs["Wl"].shape[0]
    rpc = n // ncores

    key = (n, bpc, ncores, nlayers)
    if key not in _CACHED:
        _CACHED[key] = build_program(n=n, bpc=bpc, ncores=ncores, nlayers=nlayers)
    nc = _CACHED[key]

    in_maps = prep_inputs(**inputs, ncores=ncores)
    res = run_bass_kernel_spmd(nc, in_maps, core_ids=list(range(ncores)))
    out = np.empty((n, D), np.float32)
    for c in range(ncores):
        out[c * rpc : (c + 1) * rpc, :] = res.results[c]["y"].T
    return out


if __name__ == "__main__":
    import reference

    inputs = reference.setup_inputs()
    out = kernel(**{k: np.asarray(v) for k, v in inputs.items()})
    print("out", out.shape, out.dtype)



# revision 8
# speedup vs baseline: 3.7155x; 1.2182x over previous
"""Trainium2 Bass kernel for nn_DeepSelfAttention_86346022518823.

Strategy (8 NeuronCores):
  - Shard the 200 independent QKV attention blocks 25-per-core (expert
    parallelism).  x is replicated (as x^T, bf16).
  - Per block b on each core:
      QT = Wq[b] @ x^T + bq   (computed directly transposed, [256, N])
      KT = Wk[b] @ x^T + bk
      V  = x @ Wv[b]^T        (natural layout [N, 256]; bv is skipped -- softmax
                               rows sum to 1 so the bias passes through linearly
                               and is re-added at the end as mean(bv))
      ST = KT^T @ QT          (scores *transposed*: [j, i])
      E  = exp(0.5*tanh(ST/(2*sqrt(D))) + 0.5)     == exp(sigmoid(ST/sqrt(D)))
           (tanh & exp share one ACT table set -> no table thrash)
      A[i, 0:257] += E[:, i]^T @ [V | 1]           (ones column gives the
                                                    softmax denominator free)
      acc[i, :] += A[:, :256] / A[:, 256]
  - ReduceScatter(sum) of acc over the 8 cores; each core gets a 512-row slice,
    scales by 1/200, adds mean(bv), transposes via the PE, and runs the
    20-layer MLP + final sigmoid layer on its slice in fp32.
  - Host concatenates the 8 row-slices.

All heavy matmuls are bf16 (fp32 PSUM accumulation); the MLP tail is fp32.
"""

import numpy as np
import ml_dtypes

import concourse.bass as bass
import concourse.mybir as mybir
import concourse.tile as tile
from concourse import bacc
from concourse.bass import ds
from concourse.bass_utils import run_bass_kernel_spmd
from concourse.masks import make_identity

BF16 = mybir.dt.bfloat16
F8 = mybir.dt.float8e4
F32 = mybir.dt.float32
AF = mybir.ActivationFunctionType
ALU = mybir.AluOpType
DR = mybir.MatmulPerfMode.DoubleRow

# problem constants (hardcoded per harness contract)
N = 4096
D = 256
NQKV = 200
NLAYERS = 20
NCORES = 8
P = 128
IG = 512           # i-group width (query columns per EV accumulation pass)
SGJ = 2            # j-tiles stacked per ST psum group (2 * 512 = 1024 free = 2 banks)
SCALE = 1.0 / 16.0  # 1/sqrt(D)
# exp(sigmoid(z)) ~= TA*tanh(TB*z + TC) + TD  (max rel err 4.1e-4 on [-12,12]).
# TA cancels in the softmax ratio; the TD term folds into the EV matmul as a
# rank-1 update with the column-sum of V-hat scaled by TD/TA.
TA = 0.85838554
TB = 0.50908561
TC = -0.24979685
TD = 1.85879184


def build(tc, io, n=N, bpc=NQKV // NCORES, ncores=NCORES, nlayers=NLAYERS,
          tail=True, unroll=False, timing_mode=False):
    """Emit the SPMD per-core program.  io maps tensor-name -> DRAM AP."""
    nc = tc.nc
    nqkv = bpc * ncores
    n_ic = n // P            # i-chunks (128-wide)
    n_ig = n // IG           # i-groups
    n_jt = n // P            # j-tiles
    n_sg = n_jt // SGJ       # stacked ST groups per i-group
    rpc = n // ncores        # rows per core after reduce-scatter
    n_rt = rpc // P          # 128-row tiles of the local slice
    Dp1 = D + 1

    xt_d, wq_d, wk_d, wv_d = io["xt"], io["wq"], io["wk"], io["wv"]
    bqk_d, wl_d, bld_d = io["bqk"], io["wl"], io["bld"]
    fw_d, fbias_d, meanbv_d, y_d = io["fw"], io["fbias"], io["meanbv"], io["y"]

    with (
        tc.tile_pool(name="persist", bufs=1) as persist,
        tc.tile_pool(name="dram", bufs=1, space="DRAM") as dram,
    ):
        # ---- persistent SBUF state ----
        xt0 = persist.tile([P, n], BF16)
        xt1 = persist.tile([P, n], BF16)
        # ping-pong projection state so adjacent blocks overlap (fp8 for
        # DoubleRow score / EV matmuls)
        qt_ab = [persist.tile([P, 2 * n], F8, name=f"qt{i}") for i in range(2)]
        kt_ab = [persist.tile([P, 2 * n], F8, name=f"kt{i}") for i in range(2)]
        vhat_ab = [
            persist.tile([P, n_jt * Dp1], F8, name=f"vhat{i}") for i in range(2)
        ]
        acc = persist.tile([P, n_ic * D], F32)      # (ic, d)
        id_sb = persist.tile([P, P], F32)
        meanbv_sb = persist.tile([P, 2], F32)
        fbias_sb = persist.tile([P, 2], F32)
        tc_sb = persist.tile([P, 1], F32)
        ones_col = persist.tile([P, 1], BF16)
        ones_row = persist.tile([1, P], BF16)
        nc.vector.memset(tc_sb, TC)
        nc.vector.memset(ones_col, 1.0)
        nc.vector.memset(ones_row, 1.0)

        nc.sync.dma_start(xt0, xt_d[0:P, :])
        nc.sync.dma_start(xt1, xt_d[P : 2 * P, :])
        nc.sync.dma_start(meanbv_sb, meanbv_d[:, :])
        nc.sync.dma_start(fbias_sb, fbias_d[:, :])
        make_identity(nc, id_sb)
        nc.vector.memset(acc, 0.0)
        for vh in vhat_ab:
            ones_view = vh.rearrange("p (j c) -> p j c", c=Dp1)[:, :, D : D + 1]
            nc.vector.memset(ones_view, 1.0)

        with (
            tc.tile_pool(name="work", bufs=2) as work,
            tc.tile_pool(name="ps_ev", bufs=4, space="PSUM") as ps_ev,
            tc.tile_pool(name="ps_st", bufs=2, space="PSUM") as ps_st,
        ):

            def block_body(bi, parity=0):
                qt_sb = qt_ab[parity]
                kt_sb = kt_ab[parity]
                vhat = vhat_ab[parity]
                # ---- per-block weight / bias loads ----
                if timing_mode:
                    bi = 0  # static offsets; stream shape identical
                wq_sb = work.tile([P, 2 * D], BF16, tag="wq", name="wq_sb")
                wk_sb = work.tile([P, 2 * D], BF16, tag="wk", name="wk_sb")
                wv_sb = work.tile([P, 2 * D], BF16, tag="wv", name="wv_sb")
                bqk_sb = work.tile([P, 4], F32, tag="bqk", name="bqk_sb")
                for kc in range(2):
                    nc.sync.dma_start(
                        wq_sb[:, kc * D : (kc + 1) * D], wq_d[ds(bi * D + kc * P, P), :]
                    )
                    nc.sync.dma_start(
                        wk_sb[:, kc * D : (kc + 1) * D], wk_d[ds(bi * D + kc * P, P), :]
                    )
                    nc.sync.dma_start(
                        wv_sb[:, kc * D : (kc + 1) * D], wv_d[ds(bi * D + kc * P, P), :]
                    )
                nc.sync.dma_start(bqk_sb, bqk_d[ds(bi * P, P), :])

                xts = (xt0, xt1)

                # ---- projections ----
                # Packed into ps_st-pool tiles (fast-recycling) so they never
                # contend with the long-lived EV accumulators in ps_ev.
                SGW_ = SGJ * IG
                qslots = SGW_ // IG   # QT/KT slices per psum tile
                vslots = SGW_ // D    # V slices per psum tile
                # QT / KT: [dout-chunk mc, i] = sum_kc W^T[kc, mc].T @ xT[kc]
                for w_sb, dst, qk in ((wq_sb, qt_sb, 0), (wk_sb, kt_sb, 1)):
                    for mc in range(2):
                        for icg in range(0, n_ig, qslots):
                            ps = ps_st.tile([P, SGW_], F32, tag="st", name="ps_proj")
                            cnt = min(qslots, n_ig - icg)
                            for sub in range(cnt):
                                ic = icg + sub
                                for kc in range(2):
                                    nc.tensor.matmul(
                                        ps[:, sub * IG : (sub + 1) * IG],
                                        w_sb[:, kc * D + mc * P : kc * D + mc * P + P],
                                        xts[kc][:, ic * IG : (ic + 1) * IG],
                                        start=(kc == 0),
                                        stop=(kc == 1),
                                    )
                            for sub in range(cnt):
                                ic = icg + sub
                                nc.vector.tensor_scalar(
                                    dst[:, mc * n + ic * IG : mc * n + (ic + 1) * IG],
                                    ps[:, sub * IG : (sub + 1) * IG],
                                    bqk_sb[:, qk * 2 + mc : qk * 2 + mc + 1],
                                    None,
                                    ALU.add,
                                )
                # V: [j-chunk, dout] = sum_kc xT[kc][:, jc].T @ WvT[kc]
                for jcg in range(0, n_jt, vslots):
                    ps = ps_st.tile([P, SGW_], F32, tag="st", name="ps_projv")
                    cnt = min(vslots, n_jt - jcg)
                    for sub in range(cnt):
                        jc = jcg + sub
                        for kc in range(2):
                            nc.tensor.matmul(
                                ps[:, sub * D : sub * D + D],
                                xts[kc][:, jc * P : (jc + 1) * P],
                                wv_sb[:, kc * D : (kc + 1) * D],
                                start=(kc == 0),
                                stop=(kc == 1),
                            )
                    for sub in range(cnt):
                        jc = jcg + sub
                        nc.vector.tensor_copy(
                            vhat[:, jc * Dp1 : jc * Dp1 + D],
                            ps[:, sub * D : sub * D + D],
                        )

                # svrow = (TD/TA) * sum_j vhat_j  (rank-1 EV closer term)
                sv_ps = ps_ev.tile([P, IG], F32, tag="ev", name="sv_ps")
                for jc in range(n_jt):
                    nc.tensor.matmul(
                        sv_ps[0:1, :Dp1],
                        ones_col,
                        vhat[:, jc * Dp1 : (jc + 1) * Dp1],
                        start=(jc == 0),
                        stop=(jc == n_jt - 1),
                    )
                svrow = work.tile([1, Dp1], BF16, tag="sv", name="svrow")
                nc.vector.tensor_scalar(
                    svrow, sv_ps[0:1, :Dp1], TD / TA, None, ALU.mult
                )

                # ---- attention ----
                SGW = SGJ * IG  # free width of one stacked ST group
                for g in range(n_ig):
                    evas = [
                        ps_ev.tile([P, IG], F32, tag="ev", name=f"eva{c}")
                        for c in range(4)
                    ]
                    qt3 = qt_sb.rearrange("p (k f) -> p k f", k=2)
                    kt3 = kt_sb.rearrange("p (k f) -> p k f", k=2)
                    vh3 = vhat.rearrange("p (j c) -> p j c", c=Dp1)
                    for sg in range(n_sg):
                        stp = ps_st.tile([P, SGW], F32, tag="st", name="stp")
                        for jl in range(SGJ):
                            jc = sg * SGJ + jl
                            nc.tensor.matmul(
                                stp[:, jl * IG : (jl + 1) * IG],
                                kt3[:, :, jc * P : jc * P + P],
                                qt3[:, :, g * IG : (g + 1) * IG],
                                start=True,
                                stop=True,
                                perf_mode=DR,
                            )
                        t_sb = work.tile([P, SGW], F8, tag="t", name="t_sb")
                        nc.scalar.activation(
                            t_sb, stp, AF.Tanh, bias=tc_sb, scale=TB * SCALE
                        )
                        t3 = t_sb.rearrange("p (jl f) -> p jl f", jl=2)
                        for c in range(4):
                            nc.tensor.matmul(
                                evas[c][:, :Dp1],
                                t3[:, :, c * P : (c + 1) * P],
                                vh3[:, 2 * sg : 2 * sg + 2, :],
                                start=(sg == 0),
                                stop=False,
                                perf_mode=DR,
                            )
                    for c in range(4):
                        nc.tensor.matmul(
                            evas[c][:, :Dp1],
                            ones_row,
                            svrow,
                            start=False,
                            stop=True,
                        )
                    for c in range(4):
                        gc = g * 4 + c
                        r_sb = work.tile([P, 1], F32, tag="r", name="r_sb")
                        nc.vector.reciprocal(r_sb, evas[c][:, D : D + 1])
                        tmp = work.tile([P, D], F32, tag="tmp", name="tmp")
                        nc.vector.tensor_scalar(
                            tmp, evas[c][:, :D], r_sb, None, ALU.mult
                        )
                        nc.vector.tensor_tensor(
                            acc[:, gc * D : (gc + 1) * D],
                            acc[:, gc * D : (gc + 1) * D],
                            tmp,
                            ALU.add,
                        )

            hints = (
                mybir.EngineType.PE,
                mybir.EngineType.Activation,
                mybir.EngineType.DVE,
            )
            if timing_mode:
                reps_sb = persist.tile([1, 1], mybir.dt.int32, name="reps_sb")
                nc.sync.dma_start(reps_sb, io["reps"][0:1, 0:1])
                rv = nc.values_load(
                    reps_sb[0:1, 0:1],
                    min_val=1,
                    max_val=100000,
                    skip_runtime_bounds_check=True,
                )
                with tc.For_i(0, rv, 1, hint_engines=hints) as bi:
                    block_body(bi, 0)
                    block_body(bi, 1)
            elif unroll:
                for b in range(bpc):
                    block_body(b, b % 2)
            elif bpc > 2:
                pairs = bpc // 2
                with tc.For_i(0, 2 * pairs, 2, hint_engines=hints) as bi:
                    block_body(bi, 0)
                    block_body(bi + 1, 1)
                for b in range(2 * pairs, bpc):
                    block_body(b, 0)
            else:
                for b in range(bpc):
                    block_body(b, b % 2)

        # ---- reduce-scatter over cores ----
        ar_in = dram.tile([n, D], F32, name="ar_in")
        rs_out = dram.tile([rpc, D], F32, name="rs_out")
        nc.sync.dma_start(
            ar_in[:, :].rearrange("(gc p) d -> p gc d", p=P),
            acc.rearrange("p (gc d) -> p gc d", d=D),
        )
        if not tail:
            # profiling variant: no collective / MLP; dump an acc slice as y
            nc.sync.dma_start(
                y_d[:, :].rearrange("(c p) r -> p c r", p=P),
                acc[:, : 2 * rpc].rearrange("p (c r) -> p c r", r=rpc),
            )
            return
        nc.gpsimd.collective_compute(
            "ReduceScatter",
            ALU.add,
            ins=[ar_in.opt()],
            outs=[rs_out.opt()],
            replica_groups=[list(range(ncores))],
        )

        # ---- tail: transpose slice, MLP, final layer ----
        with (
            tc.tile_pool(name="tail", bufs=2) as tail,
            tc.tile_pool(name="ps_tail", bufs=4, space="PSUM") as ps_tail,
        ):
            rs_sb = tail.tile([P, n_rt * D], F32, name="rs_sb", bufs=1)
            nc.sync.dma_start(
                rs_sb.rearrange("p (t d) -> p t d", d=D),
                rs_out[:, :].rearrange("(t p) d -> p t d", p=P),
            )
            ht_a = [
                tail.tile([P, rpc], F32, name=f"hta{dc}", bufs=1) for dc in range(2)
            ]
            ht_b = [
                tail.tile([P, rpc], F32, name=f"htb{dc}", bufs=1) for dc in range(2)
            ]
            for dc in range(2):
                for t in range(n_rt):
                    pst = ps_tail.tile([P, IG], F32, tag="pt", name="pst")
                    nc.tensor.transpose(
                        pst[:, :P], rs_sb[:, t * D + dc * P : t * D + dc * P + P], id_sb
                    )
                    nc.vector.tensor_scalar(
                        ht_a[dc][:, t * P : (t + 1) * P],
                        pst[:, :P],
                        1.0 / nqkv,
                        meanbv_sb[:, dc : dc + 1],
                        ALU.mult,
                        ALU.add,
                    )
            h_cur, h_nxt = ht_a, ht_b
            for l in range(nlayers):
                wl_sb = tail.tile([P, 2 * D], F32, tag="wl", name="wl_sb")
                bl_sb = tail.tile([P, 2], F32, tag="bl", name="bl_sb")
                for kc in range(2):
                    nc.sync.dma_start(
                        wl_sb[:, kc * D : (kc + 1) * D],
                        wl_d[l * D + kc * P : l * D + kc * P + P, :],
                    )
                nc.sync.dma_start(bl_sb, bld_d[l * P : (l + 1) * P, :])
                for dc in range(2):
                    ps = ps_tail.tile([P, IG], F32, tag="pt", name="ps_mlp")
                    for kc in range(2):
                        nc.tensor.matmul(
                            ps[:, :rpc],
                            wl_sb[:, kc * D + dc * P : kc * D + dc * P + P],
                            h_cur[kc],
                            start=(kc == 0),
                            stop=(kc == 1),
                        )
                    nc.vector.tensor_scalar(
                        h_nxt[dc],
                        ps[:, :rpc],
                        bl_sb[:, dc : dc + 1],
                        0.0,
                        ALU.add,
                        ALU.max,
                    )
                h_cur, h_nxt = h_nxt, h_cur
            # final: y = sigmoid(h^T @ fw)^T + fb
            fw_sb = tail.tile([P, 2 * D], F32, name="fw_sb", bufs=1)
            for kc in range(2):
                nc.sync.dma_start(
                    fw_sb[:, kc * D : (kc + 1) * D], fw_d[kc * P : (kc + 1) * P, :]
                )
            for dc in range(2):
                ps = ps_tail.tile([P, IG], F32, tag="pt", name="ps_fin")
                for kc in range(2):
                    nc.tensor.matmul(
                        ps[:, :rpc],
                        fw_sb[:, kc * D + dc * P : kc * D + dc * P + P],
                        h_cur[kc],
                        start=(kc == 0),
                        stop=(kc == 1),
                    )
                sig = tail.tile([P, rpc], F32, tag="sig", name="sig")
                nc.scalar.activation(sig, ps[:, :rpc], AF.Sigmoid)
                y_sb = tail.tile([P, rpc], F32, tag="ysb", name="y_sb")
                nc.vector.tensor_scalar(
                    y_sb, sig, fbias_sb[:, dc : dc + 1], None, ALU.add
                )
                nc.sync.dma_start(y_d[dc * P : (dc + 1) * P, :], y_sb)


def prep_inputs(x, Wq, bq, Wk, bk, Wv, bv, Wl, bl, final_weight, final_bias,
                ncores=NCORES):
    """Host-side sharding / layout prep.  Returns (in_maps, meta)."""
    bf16 = ml_dtypes.bfloat16
    n, d = x.shape
    nqkv = Wq.shape[0]
    bpc = nqkv // ncores
    nlayers = Wl.shape[0]

    xt = np.ascontiguousarray(x.T).astype(bf16)                    # [D, N]
    wl_t = np.ascontiguousarray(Wl.transpose(0, 2, 1)).reshape(nlayers * d, d).astype(np.float32)
    bld = np.ascontiguousarray(bl.reshape(nlayers, 2, P).transpose(0, 2, 1)).reshape(nlayers * P, 2).astype(np.float32)
    fw = np.ascontiguousarray(final_weight).astype(np.float32)     # [D, D]
    fbias = np.ascontiguousarray(final_bias.reshape(2, P).T).astype(np.float32)
    meanbv = np.ascontiguousarray(bv.mean(axis=0).reshape(2, P).T).astype(np.float32)

    in_maps = []
    for c in range(ncores):
        sl = slice(c * bpc, (c + 1) * bpc)
        wq_t = np.ascontiguousarray(Wq[sl].transpose(0, 2, 1)).reshape(bpc * d, d).astype(bf16)
        wk_t = np.ascontiguousarray(Wk[sl].transpose(0, 2, 1)).reshape(bpc * d, d).astype(bf16)
        wv_t = np.ascontiguousarray(Wv[sl].transpose(0, 2, 1)).reshape(bpc * d, d).astype(bf16)
        # bqk[b*P + p, qk*2 + mc] = bias[qk][b, mc*P + p]
        bqk = np.empty((bpc * P, 4), np.float32)
        for qk, arr in ((0, bq[sl]), (1, bk[sl])):
            r = arr.reshape(bpc, 2, P)                             # [b, mc, p]
            bqk[:, qk * 2 : qk * 2 + 2] = r.transpose(0, 2, 1).reshape(bpc * P, 2)
        in_maps.append({
            "xt": xt, "wq": wq_t, "wk": wk_t, "wv": wv_t, "bqk": bqk,
            "wl": wl_t, "bld": bld, "fw": fw, "fbias": fbias, "meanbv": meanbv,
        })
    return in_maps


def build_program(n=N, bpc=NQKV // NCORES, ncores=NCORES, nlayers=NLAYERS,
                  tail=True, unroll=False, timing_mode=False):
    nc = bacc.Bacc("TRN2", target_bir_lowering=False, debug=False,
                   num_devices=ncores)
    d = D
    rpc = n // ncores
    io = {
        "xt": nc.dram_tensor("xt", [d, n], BF16, kind="ExternalInput").ap(),
        "wq": nc.dram_tensor("wq", [bpc * d, d], BF16, kind="ExternalInput").ap(),
        "wk": nc.dram_tensor("wk", [bpc * d, d], BF16, kind="ExternalInput").ap(),
        "wv": nc.dram_tensor("wv", [bpc * d, d], BF16, kind="ExternalInput").ap(),
        "bqk": nc.dram_tensor("bqk", [bpc * P, 4], F32, kind="ExternalInput").ap(),
        "wl": nc.dram_tensor("wl", [nlayers * d, d], F32, kind="ExternalInput").ap(),
        "bld": nc.dram_tensor("bld", [nlayers * P, 2], F32, kind="ExternalInput").ap(),
        "fw": nc.dram_tensor("fw", [d, d], F32, kind="ExternalInput").ap(),
        "fbias": nc.dram_tensor("fbias", [P, 2], F32, kind="ExternalInput").ap(),
        "meanbv": nc.dram_tensor("meanbv", [P, 2], F32, kind="ExternalInput").ap(),
        "y": nc.dram_tensor("y", [d, rpc], F32, kind="ExternalOutput").ap(),
    }
    if timing_mode:
        io["reps"] = nc.dram_tensor(
            "reps", [1, 1], mybir.dt.int32, kind="ExternalInput"
        ).ap()
    with tile.TileContext(nc) as tc:
        build(tc, io, n=n, bpc=bpc, ncores=ncores, nlayers=nlayers, tail=tail,
              unroll=unroll, timing_mode=timing_mode)
    nc.compile()
    return nc


_CACHED = {}


def kernel(**inputs):
    inputs = {k: np.asarray(v) for k, v in inputs.items()}
    x = inputs["x"]
    n = x.shape[0]
    nqkv = inputs["Wq"].shape[0]
    ncores = NCORES
    bpc = nqkv // ncores
    nlayers = inputs["Wl"].shape[0]
    rpc = n // ncores

    key = (n, bpc, ncores, nlayers)
    if key not in _CACHED:
        _CACHED[key] = build_program(n=n, bpc=bpc, ncores=ncores, nlayers=nlayers)
    nc = _CACHED[key]

    in_maps = prep_inputs(**inputs, ncores=ncores)
    res = run_bass_kernel_spmd(nc, in_maps, core_ids=list(range(ncores)))
    out = np.empty((n, D), np.float32)
    for c in range(ncores):
        out[c * rpc : (c + 1) * rpc, :] = res.results[c]["y"].T
    return out


if __name__ == "__main__":
    import reference

    inputs = reference.setup_inputs()
    out = kernel(**{k: np.asarray(v) for k, v in inputs.items()})
    print("out", out.shape, out.dtype)

